# revision 41
# baseline (speedup 1.0000x reference)
"""Masked attention kernel for Trainium2, 8 NeuronCores.

Problem: q,k,v [32,1024,64] f32, mask [32,1024,1024] bool (True -> -inf),
out = softmax(q@k^T * D^-0.5 masked) @ v.

Sharding: batch*head dim (32) split across 8 cores, 4 heads/core.

Per-core device algorithm (T-layout):
  scoresT[t,s] = sum_d k[t,d] q[s,d]  computed via PE with
      lhsT = kT [64,128] chunk, rhs = qT [64,1024]  (host provides qT/kT)
  pT = exp(0.125 * scoresT)  on ACT (no row-max needed: |s|*0.125 <= ~6)
  mask applied POST-exp as p *= mkeep (mkeep in {1,0} fp8) on DVE (tiles
      0-5) and Pool (tiles 6-7) so the PE never sees the mask at all.
  outT_aug[d,s] = sum_t v_aug[t,d] pT[t,s]  with v_aug = [v | ones | pad]
      so row 64 carries the softmax denominators; computed in two s-halves
      of 512 so each PSUM accumulator is a single bank.
  tail: outT halves are PE-transposed back to [s,d] with the denominator row
      riding along (identity65), reciprocal runs 128 lanes wide on the
      transposed denominators, division is a broadcast tensor_mul, output
      leaves as bf16 (host casts back to f32).
QK runs as bf16 row-group pairs (qT/kT duplicated into partitions 64-127;
only head 0 ships pre-duplicated from HBM, later heads duplicate via an
SBUF->SBUF DMA) so two tau-tiles stream through the PE concurrently.
Scheduling: the av chunks of head h-1 are woven one-by-one between head h's
QK pairs (head 0 self-weaves, two tiles behind its own mask) so the PE has
dependency-free work during the exp->PSUM-bank-recycle interlock; the
per-half PSUM->SBUF casts go to DVE (half A) and ACT (half B) so the two
accumulator banks recycle independently. All DRAM tensors are host
pre-tiled so every DMA is a straight contiguous copy. Host does only
pure-layout work: transposes/casts/tiling of inputs+outputs.
"""

import os
import sys

import numpy as np

for _p in ("/opt/trn_rl_repo", "/opt/pypackages"):
    if _p not in sys.path and os.path.isdir(_p):
        sys.path.append(_p)

import ml_dtypes  # noqa: E402

import concourse.bass as bass  # noqa: E402
import concourse.tile as tile  # noqa: E402
from concourse import mybir  # noqa: E402
from concourse.bass_utils import run_bass_kernel_spmd  # noqa: E402

BH, S, D = 32, 1024, 64
NCORES = 8
HPC = BH // NCORES  # heads per core
NT = S // 128  # 8 tiles of 128 along s/t
FP8 = ml_dtypes.float8_e4m3fn
F32 = mybir.dt.float32
BF16 = mybir.dt.bfloat16
DT8 = mybir.dt.float8e4


def _build_program():
    nc = bass.Bass(
        "TRN2",
        target_bir_lowering=False,
        debug=False,
        num_devices=NCORES,
    )
    qkt = nc.dram_tensor("qkt", [HPC, 128, 2 * S], BF16, kind="ExternalInput").ap()
    qk1t = nc.dram_tensor("qk1t", [HPC, 64, 2 * S], BF16, kind="ExternalInput").ap()
    vaug = nc.dram_tensor("vaug", [HPC, 128, NT * 80], BF16, kind="ExternalInput").ap()
    mk8 = nc.dram_tensor("mk8", [HPC, 128, NT * S], DT8, kind="ExternalInput").ap()
    ident = nc.dram_tensor("ident", [65, 65], BF16, kind="ExternalInput").ap()
    outp = nc.dram_tensor("outp", [HPC, 128, NT * D], BF16, kind="ExternalOutput").ap()

    with tile.TileContext(nc) as tc:
        with (
            tc.tile_pool(name="const", bufs=1) as const_pool,
            tc.tile_pool(name="qk", bufs=HPC) as qk_pool,
            tc.tile_pool(name="v", bufs=HPC) as v_pool,
            tc.tile_pool(name="m", bufs=HPC) as m_pool,
            tc.tile_pool(name="p", bufs=3) as p_pool,
            tc.tile_pool(name="ot", bufs=5) as ot_pool,
            tc.tile_pool(name="fin", bufs=2) as fin_pool,
            tc.tile_pool(name="spsum", bufs=2, space="PSUM") as s_pool,
            tc.tile_pool(name="opsum", bufs=2, space="PSUM") as o_pool,
            tc.tile_pool(name="fpsum", bufs=2, space="PSUM") as f_pool,
        ):
            ident_sb = const_pool.tile([65, 65], BF16)
            nc.sync.dma_start(ident_sb[:], ident[:])
            warm_sb = const_pool.tile([1, 1], F32)
            nc.gpsimd.memset(warm_sb[:], 0.0)
            warm_out = const_pool.tile([1, 1], F32, tag="warmo")
            nc.scalar.activation(
                out=warm_out[:],
                in_=warm_sb[:],
                func=mybir.ActivationFunctionType.Exp,
            )

            # DMA staging: qkt for head 0 is loaded pre-duplicated from HBM
            # (fast start); later heads load the 64-row original and an
            # SBUF->SBUF DMA duplicates it into partitions 64-127, halving
            # HBM traffic for q/k.  Mask tiles stream in consumption order.
            qk_tiles, v_tiles, m_tiles = [], [], []
            for h in range(HPC):
                qk_tiles.append(qk_pool.tile([128, 2 * S], BF16, name="qk_sb"))
                m_tiles.append(m_pool.tile([128, NT * S], DT8, name="m_sb"))
                v_tiles.append(v_pool.tile([128, NT * 80], BF16, name="v_sb"))

            def load_qk(h):
                if h == 0:
                    nc.sync.dma_start(
                        qk_tiles[0][:, : S + 256], qkt[0][:, : S + 256]
                    )
                    nc.sync.dma_start(
                        qk_tiles[0][:, S + 256 :], qkt[0][:, S + 256 :]
                    )
                else:
                    nc.sync.dma_start(qk_tiles[h][0:64, :], qk1t[h])
                    nc.sync.dma_start(qk_tiles[h][64:128, :], qk_tiles[h][0:64, :])

            def load_m(h, lo, hi):
                nc.sync.dma_start(
                    m_tiles[h][:, lo * S : hi * S], mk8[h][:, lo * S : hi * S]
                )

            def load_v(h):
                nc.sync.dma_start(v_tiles[h][:], vaug[h])

            load_qk(0)
            load_m(0, 0, 1)
            load_qk(1)
            load_v(0)
            load_m(0, 1, 8)
            load_qk(2)
            load_m(1, 0, 4)
            load_qk(3)
            load_v(1)
            load_m(1, 4, 8)
            load_m(2, 0, 4)
            load_v(2)
            load_m(2, 4, 8)
            load_m(3, 0, 4)
            load_v(3)
            load_m(3, 4, 8)

            p_tiles = {}

            # mask multiply engine per tile: DVE handles tiles 0-5, Pool 6-7
            # (DVE ~1.4us vs Pool ~2.4us per 1024-wide tile; Pool tiles are
            # the last consumed by each AV half, and tiles self-woven into
            # scores(0) must be DVE-masked to avoid stalling the PE queue)
            def mask_eng(t, h):
                # DVE (fast) takes tile 7 as well: it is the last tile the
                # AV weave consumes, and Pool's serial 2.4us/tile chain
                # otherwise delivers it after the av chunks want it
                if t <= 4 or t == 7:
                    return nc.vector
                if t == 5 and h == 0:
                    # head 0 self-weaves tile 5 early; DVE masks it faster
                    return nc.vector
                return nc.gpsimd

            o_halves = {}
            av_state = {}

            def start_av(h):
                # two accumulation halves (one PSUM bank each) fed chunk by
                # chunk from the weave below
                oa = o_pool.tile([80, 512], F32, tag="ops")
                ob = o_pool.tile([80, 512], F32, tag="ops")
                av_state[h] = (oa, ob)

            def emit_av_chunk(h, half, t):
                o_ps = av_state[h][half]
                nc.tensor.matmul(
                    out=o_ps[:],
                    lhsT=v_tiles[h][:, t * 80 : (t + 1) * 80],
                    rhs=p_tiles[h][
                        :, t * S + half * 512 : t * S + half * 512 + 512
                    ],
                    start=(t == 0),
                    stop=(t == NT - 1),
                )
                if t == NT - 1:
                    ot_sb = ot_pool.tile([80, 512], BF16, name="ot_sb")
                    # PSUM->SBUF cast: DVE for half A, ACT for half B (GPSIMD
                    # cannot read PSUM), so each o_ps bank recycles without
                    # queuing behind the other engine
                    if half == 0:
                        nc.vector.tensor_copy(ot_sb[:], o_ps[:])
                    else:
                        nc.scalar.activation(
                            out=ot_sb[:],
                            in_=o_ps[:],
                            func=mybir.ActivationFunctionType.Copy,
                        )
                    o_halves.setdefault(h, []).append(ot_sb)

            def emit_head(h):
                """Scores of head h with av chunks of head h-1 woven between
                QK pairs, so the PE has dependency-free work during the
                exp->PSUM-recycle interlock."""
                qk_sb, m_sb = qk_tiles[h], m_tiles[h]
                p_sb = p_pool.tile([128, NT * S], BF16, name="p_sb")
                p_tiles[h] = p_sb
                prev = h - 1 if h >= 1 else None
                if h == 0:
                    start_av(0)
                elif h >= 2:
                    start_av(prev)
                for pi, t0 in enumerate((0, 2, 4, 6)):
                    # two tau-tiles run concurrently in disjoint PE row groups
                    # (qT/kT duplicated into partitions 64-127)
                    tiles2 = []
                    for i, rows in ((0, slice(0, 64)), (1, slice(64, 128))):
                        t = t0 + i
                        s_ps = s_pool.tile([128, S], F32, tag="sps")
                        kslc = slice(S + t * 128, S + (t + 1) * 128)
                        for n in range(2):
                            sl = slice(n * 512, (n + 1) * 512)
                            nc.tensor.matmul(
                                out=s_ps[:, sl],
                                lhsT=qk_sb[rows, kslc],
                                rhs=qk_sb[rows, sl],
                                start=True,
                                stop=True,
                            )
                        tiles2.append((t, s_ps))
                    for t, s_ps in tiles2:
                        nc.scalar.activation(
                            out=p_sb[:, t * S : (t + 1) * S],
                            in_=s_ps[:],
                            func=mybir.ActivationFunctionType.Exp,
                            scale=0.125,
                        )
                        csl = slice(t * S, (t + 1) * S)
                        mask_eng(t, h).tensor_mul(
                            out=p_sb[:, csl],
                            in0=p_sb[:, csl],
                            in1=m_sb[:, csl],
                        )
                    if h == 0:
                        # first head: self-weave av(0) two tiles behind the
                        # mask (fresh PSUM banks, so no recycle deadlock)
                        if pi >= 1:
                            for tt in (2 * pi - 2, 2 * pi - 1):
                                emit_av_chunk(0, 0, tt)
                                emit_av_chunk(0, 1, tt)
                    elif h == 1:
                        # av(0) remainder (tiles 6,7); av(1) can't weave yet
                        if pi == 0:
                            for tt in (6, 7):
                                emit_av_chunk(0, 0, tt)
                                emit_av_chunk(0, 1, tt)
                    else:
                        for tt in (2 * pi, 2 * pi + 1):
                            emit_av_chunk(prev, 0, tt)
                            emit_av_chunk(prev, 1, tt)
                if h >= 2:
                    # tail of head h-2: its casts finished during scores(h-1),
                    # so the transpose weight-loads never stall the PE queue
                    emit_tail(h - 2)

            def emit_half_tail(h, half, ot_sb, f_ps):
                # transpose one s-half back to [s,d] with the denominator row
                # riding along as column 64 of each block (66-wide blocks
                # keep every PSUM access 4-byte aligned for bf16)
                for jj in range(4):
                    j = half * 4 + jj
                    nc.tensor.transpose(
                        out=f_ps[:, j * 66 : j * 66 + 65],
                        in_=ot_sb[0:65, jj * 128 : (jj + 1) * 128],
                        identity=ident_sb[:],
                    )
                fv = f_ps[:].rearrange("p (j c) -> p j c", j=8)[
                    :, half * 4 : half * 4 + 4
                ]
                r_sb = fin_pool.tile([128, 4], F32, tag="rsb")
                nc.vector.reciprocal(r_sb[:, :, None], fv[:, :, 64:65])
                out_sb = fin_pool.tile([128, 4 * D], BF16, tag="osb")
                nc.vector.tensor_mul(
                    out=out_sb[:].rearrange("p (j d) -> p j d", j=4),
                    in0=fv[:, :, 0:64],
                    in1=r_sb[:, :, None].to_broadcast((128, 4, D)),
                )
                nc.sync.dma_start(
                    outp[h][:, half * 256 : half * 256 + 256], out_sb[:]
                )

            def emit_tail(h):
                p_tiles.pop(h)
                ot_sbs = o_halves.pop(h)
                f_ps = f_pool.tile([128, 8 * 66], BF16)
                for j in range(8):
                    nc.tensor.transpose(
                        out=f_ps[:, j * 66 : j * 66 + 65],
                        in_=ot_sbs[j // 4][0:65, (j % 4) * 128 : (j % 4 + 1) * 128],
                        identity=ident_sb[:],
                    )
                fv = f_ps[:].rearrange("p (j c) -> p j c", j=8)
                r_sb = fin_pool.tile([128, 8], F32, tag="rsb8")
                nc.vector.reciprocal(r_sb[:, :, None], fv[:, :, 64:65])
                out_sb = fin_pool.tile([128, 8 * D], BF16, tag="osb8")
                nc.vector.tensor_mul(
                    out=out_sb[:].rearrange("p (j d) -> p j d", j=8),
                    in0=fv[:, :, 0:64],
                    in1=r_sb[:, :, None].to_broadcast((128, 8, D)),
                )
                nc.sync.dma_start(outp[h], out_sb[:])

            for h in range(HPC):
                emit_head(h)
            # last head's av has no next scores to weave into; half A's tail
            # overlaps the final B chunks
            last = HPC - 1
            start_av(last)
            for t in range(NT):
                emit_av_chunk(last, 0, t)
                emit_av_chunk(last, 1, t)
                if t == 1:
                    emit_tail(HPC - 2)
            f_ps_last = f_pool.tile([128, 8 * 66], BF16, name="f_ps")
            emit_half_tail(last, 0, o_halves[last][0], f_ps_last)
            emit_half_tail(last, 1, o_halves[last][1], f_ps_last)
            p_tiles.pop(last)
            o_halves.pop(last)

    _split_multi_waits(nc)
    return nc


def _split_multi_waits(nc):
    """Walrus's S3_LW codegen can't take >1 sync-wait condition on a Matmult;
    hoist extras into standalone EventSemaphore instructions (same semantics:
    the engine queue stalls on them in program order, like raw-bass wait_ge).

    Before splitting, drop subsumed waits: engine queues execute in program
    order and tile semaphores only count up, so a wait sem>=Y after an
    earlier wait sem>=X (X>=Y) on the same engine is a no-op."""
    for bb in nc.bb_map.values():
        insts = bb.bb.instructions
        seen: dict = {}
        for inst in insts:
            si = getattr(inst, "sync_info", None)
            if si is None or not si.on_wait:
                continue
            eng = getattr(inst, "engine", None)
            e_seen = seen.setdefault(eng, {})
            kept = []
            for cond in si.on_wait:
                if cond.wait_mode == "sem-ge-imm":
                    prev = e_seen.get(cond.id)
                    if prev is not None and prev >= cond.wait_value:
                        continue
                    e_seen[cond.id] = max(prev or 0, cond.wait_value)
                else:
                    # non-monotone wait: stop tracking this semaphore
                    e_seen.pop(cond.id, None)
                kept.append(cond)
            si.on_wait = kept
    for bb in nc.bb_map.values():
        insts = bb.bb.instructions
        new_list = []
        for inst in insts:
            si = getattr(inst, "sync_info", None)
            if (
                si is not None
                and si.on_wait
                and len(si.on_wait) > 1
            ):
                extra = si.on_wait[:-1]
                keep = si.on_wait[-1:]
                for cond in extra:
                    new_list.append(
                        mybir.InstEventSemaphore(
                            name=nc.get_next_instruction_name(),
                            ins=[],
                            outs=[],
                            engine=inst.engine,
                            sync_info=mybir.SyncInfo(on_wait=[cond], on_update=[]),
                        )
                    )
                si.on_wait = keep
            new_list.append(inst)
        insts[:] = new_list


import concourse.bass_utils as _bu

_orig_run_command = _bu.run_command


def _run_command_ldwopt(cmd, **kw):
    if os.environ.get("LDW_OPT") == "1":
        cmd = [
            "--enable-ldw-opt=true" if c == "--enable-ldw-opt=false" else c
            for c in cmd
        ]
    return _orig_run_command(cmd, **kw)


_bu.run_command = _run_command_ldwopt

_NC_CACHE = None


def _get_nc():
    global _NC_CACHE
    if _NC_CACHE is None:
        _NC_CACHE = _build_program()
    return _NC_CACHE


def _make_in_maps(q, k, v, mask):
    q = np.ascontiguousarray(np.asarray(q, dtype=np.float32))
    k = np.ascontiguousarray(np.asarray(k, dtype=np.float32))
    v = np.ascontiguousarray(np.asarray(v, dtype=np.float32))
    mask = np.asarray(mask)
    ident_np = np.eye(65, dtype=ml_dtypes.bfloat16)
    ones_col = np.ones((HPC, S, 1), dtype=np.float32)
    in_maps = []
    for c in range(NCORES):
        sl = slice(c * HPC, (c + 1) * HPC)
        qT = q[sl].transpose(0, 2, 1)  # [HPC, 64, S]
        kT = k[sl].transpose(0, 2, 1)
        qk1 = np.concatenate([qT, kT], axis=2)  # [HPC, 64, 2S]
        qkt_np = np.ascontiguousarray(
            np.concatenate([qk1, qk1], axis=1)
        ).astype(ml_dtypes.bfloat16)  # rows duplicated for PE row-group packing
        va = np.concatenate(
            [v[sl], ones_col, np.zeros((HPC, S, 15), np.float32)], axis=2
        )  # [HPC, S, 80]: 64 dims + denominator ones + pad to 80 for XBAR
        vaug_np = np.ascontiguousarray(
            va.reshape(HPC, NT, 128, 80).transpose(0, 2, 1, 3).reshape(HPC, 128, NT * 80)
        ).astype(ml_dtypes.bfloat16)
        # keep-multiplicand: 1.0 where attended, 0.0 where masked out
        mkT = (~mask[sl]).transpose(0, 2, 1).astype(np.float32)  # [HPC, t=S, s=S]
        mk8_np = np.ascontiguousarray(
            mkT.reshape(HPC, NT, 128, S).transpose(0, 2, 1, 3).reshape(HPC, 128, NT * S)
        ).astype(FP8)
        in_maps.append(
            {
                "qkt": qkt_np,
                "qk1t": np.ascontiguousarray(qk1).astype(ml_dtypes.bfloat16),
                "vaug": vaug_np,
                "mk8": mk8_np,
                "ident": ident_np,
            }
        )
    return in_maps


def _gather(results):
    outs = []
    for c in range(NCORES):
        o = np.asarray(results[c]["outp"], dtype=np.float32)  # [HPC,128,NT*D]
        o = o.reshape(HPC, 128, NT, D).transpose(0, 2, 1, 3).reshape(HPC, S, D)
        outs.append(o)
    return np.ascontiguousarray(np.concatenate(outs, axis=0))


def _install_profile_shim():
    """The agent image's antenv lacks axon_hooks; recreate it from the boot
    module's ctypes implementation so trace=True can capture NTFF profiles."""
    import types

    if "antenv.axon_hooks" in sys.modules:
        return
    try:
        from trn_agent_boot.trn_boot import _ntff_profile_via_ctypes

        hook = _ntff_profile_via_ctypes("/opt/axon/libaxon_pjrt.so")
        mod = types.ModuleType("antenv.axon_hooks")
        mod.get_axon_ntff_profile_hook = lambda: hook
        mod.set_axon_ntff_profile_hook = lambda h: None
        sys.modules["antenv.axon_hooks"] = mod
        # don't try to copy artifacts to a remote bucket from the sandbox
        import concourse.bass_utils as _bu

        _bu.upload_artifacts = lambda tmpdir: tmpdir
    except Exception as e:  # profiling is best-effort
        print(f"profile shim unavailable: {e}", file=sys.stderr)


def run(q, k, v, mask, trace=False, **kw):
    nc = _get_nc()
    if trace:
        _install_profile_shim()
    in_maps = _make_in_maps(q, k, v, mask)
    res = run_bass_kernel_spmd(nc, in_maps, list(range(NCORES)), trace=trace, **kw)
    return _gather(res.results), res


def kernel(q, k, v, mask):
    out, _ = run(q, k, v, mask)
    return out
